# revision 17
# baseline (speedup 1.0000x reference)
"""Trainium2 Bass kernel for CrossAttention2d.

Reference computation (per batch b):
    q = conv_feat[b] (as [C, HW]) projected -> [HW, d], + q_b
    k, v = vit_feat[b] [N, D] projected -> [N, d], + biases
    attn = softmax(q @ k.T / sqrt(d))          [HW, N]
    o = attn @ v                               [HW, d]
    out = o @ out_w.T + out_b -> [C, HW]

Sharding: data-parallel over batch B=8 across the 8 NeuronCores; each core
computes one full batch element.

Bias folding (exact):
  - k_b: (q+q_b).k_b is constant along the key axis -> cancels in softmax;
    k_b is never loaded.
  - q_b: logit = q_raw.K + (q_b.K); the latter depends only on the key, so
    it becomes the per-partition bias of the Exp activation:
    E = exp(S*0.125 + kqb*0.125), kqb[n] = sum_d K^T[d,n] q_b[d].
  - v_b: attn rows sum to 1 so attn@(V+vb) = attn@V + vb; folded into the
    output bias ob2 = out_b + out_w @ v_b.

Engine plan (per core):
  DMA   : weights+row-dups+outputs on sync (SP HWDGE); conv fp32 on scalar
          (ACT HWDGE); vit on gpsimd (SWDGE fp32->bf16 cast). 3 queues run
          in parallel.
  PE    : Q/S matmuls run in f32r straight off the fp32 conv data (1 cyc/row
          at N=512, no cast needed); K/V/O/out-proj in bf16.  S (K=64) and
          the two K=64 halves of each O matmul alternate PE row groups
          (tile_position) so adjacent matmuls execute concurrently.
  ACT   : only Exp (the steady-state bottleneck) + a few head copies.
  DVE   : PSUM->SBUF copies, reciprocal, normalize multiply, bias add.
  Pool  : SWDGE descriptor gen + per-block denominator broadcast.

Steady state: 8 query blocks of 512; block g's O accumulation interleaves
with block g+1's S/Exp, and block g-1's normalize/out-project/store tail is
scattered into the c-slots so no engine stalls.
"""

import numpy as np

B = 8
C = 256
H = W = 64
HW = 4096
N = 1024
D = 768
d = 64
GQ = 512           # queries per steady-state block
NG = HW // GQ      # 8
CH = N // 128      # 8 key chunks

PACK_S = False      # duplicate kT/qT rows 64-127; alternate S row groups
USE_F32R = False    # f32r Q-side matmuls (no input cast) vs bf16
SPLIT_O = False     # split O matmuls into two K=64 row-group halves

_CACHED_NC = None
_DBG_PARAMS = {}


def _build_nc():
    import os
    import concourse.mybir as mybir
    from concourse import bacc
    from concourse.masks import make_identity
    from concourse.tile import TileContext

    dt = mybir.dt
    f32 = dt.float32
    f32r = dt.float32r
    bf16 = dt.bfloat16
    Exp = mybir.ActivationFunctionType.Exp
    Copy = mybir.ActivationFunctionType.Copy
    mult = mybir.AluOpType.mult
    add = mybir.AluOpType.add

    nc = bacc.Bacc(None)

    conv = nc.declare_dram_parameter("conv_feat", [C, HW], f32, isOutput=False)
    vit = nc.declare_dram_parameter("vit_feat", [N, D], f32, isOutput=False)
    q_w = nc.declare_dram_parameter("q_w", [d, C], f32, isOutput=False)
    q_b = nc.declare_dram_parameter("q_b", [d], f32, isOutput=False)
    k_w = nc.declare_dram_parameter("k_w", [d, D], f32, isOutput=False)
    v_w = nc.declare_dram_parameter("v_w", [d, D], f32, isOutput=False)
    v_b = nc.declare_dram_parameter("v_b", [d], f32, isOutput=False)
    out_w = nc.declare_dram_parameter("out_w", [C, d], f32, isOutput=False)
    out_b = nc.declare_dram_parameter("out_b", [C], f32, isOutput=False)
    out = nc.declare_dram_parameter("out", [C, HW], f32, isOutput=True)
    if os.environ.get("BASS_DEBUG_TAPS") == "1":
        for tname, tshape in [
            ("dbg_wqT", [128, 2, d]), ("dbg_wkT", [128, 6, d]),
            ("dbg_woT", [d, 2, 128]), ("dbg_qT", [64, HW]),
            ("dbg_kT", [64, N]), ("dbg_vT", [d, N]),
            ("dbg_v", [128, CH, 65]), ("dbg_kqb", [128, 16]),
            ("dbg_ob2", [128, 2]), ("dbg_e0", [128, CH, GQ]),
            ("dbg_vitT", [128, 6, N]),
            ("dbg_o0", [65, GQ]), ("dbg_o7", [65, GQ]),
            ("dbg_r7", [1, GQ]), ("dbg_rb7", [64, GQ]),
            ("dbg_ot7", [64, GQ]), ("dbg_osb7", [128, 2, GQ]),
        ]:
            globals()  # no-op
            taps_decl = nc.declare_dram_parameter(tname, tshape, f32, isOutput=True)
            _DBG_PARAMS[tname] = taps_decl

    QROWS = 128 if PACK_S else 64
    qdt = mybir.dt.float32r if USE_F32R else mybir.dt.bfloat16
    phase = os.environ.get("BUILD_PHASE", "full")
    taps_on = os.environ.get("BASS_DEBUG_TAPS") == "1"
    taps = {}

    with TileContext(nc) as tc:
        with (
            tc.tile_pool(name="const", bufs=1) as const,
            tc.tile_pool(name="data", bufs=1) as data,
            tc.tile_pool(name="epool", bufs=2) as epool,
            tc.tile_pool(name="opool", bufs=2) as opool,
            tc.tile_pool(name="work", bufs=2) as work,
            tc.tile_pool(name="pqp", bufs=2, space="PSUM") as pqp,
        ):
            # ---- identities ------------------------------------------------
            idf = const.tile([128, 128], f32)
            make_identity(nc, idf)
            idb = const.tile([128, 128], bf16)
            make_identity(nc, idb)

            # ---- weight loads: sync queue, fp32 ---------------------------
            wq_raw = const.tile([d, C], f32)
            nc.sync.dma_start(wq_raw, q_w[:, :])
            wk_raw = const.tile([d, D], f32)
            nc.sync.dma_start(wk_raw, k_w[:, :])
            wv_raw = const.tile([d, D], f32)
            nc.sync.dma_start(wv_raw, v_w[:, :])
            wo_raw = const.tile([128, 2, d], f32)
            nc.sync.dma_start(wo_raw, out_w.rearrange("(t p) e -> p t e", p=128))
            qb_sb = const.tile([d, 1], f32)
            nc.sync.dma_start(qb_sb, q_b.rearrange("(a b) -> a b", b=1))
            vb_sb = const.tile([d, 1], f32)
            nc.sync.dma_start(vb_sb, v_b.rearrange("(a b) -> a b", b=1))
            ob_sb = const.tile([128, 2], f32)
            nc.sync.dma_start(ob_sb, out_b.rearrange("(t p) -> p t", p=128))

            # ---- input loads ----------------------------------------------
            # vit: SWDGE (gpsimd) with fp32->bf16 cast
            vit_sb = data.tile([128, 8, D], bf16)
            vit_r = vit.rearrange("(c p) e -> p c e", p=128)
            nc.gpsimd.dma_start(vit_sb[:, 0:4, :], vit_r[:, 0:4, :])
            nc.gpsimd.dma_start(vit_sb[:, 4:8, :], vit_r[:, 4:8, :])
            # conv: scalar-queue HWDGE, fp32 (used as f32r by the PE)
            conv_sb = data.tile([128, 2, HW], qdt)
            conv_r = conv.rearrange("(t p) f -> p t f", p=128)
            for jp in range(4):
                sl = slice(jp * 1024, (jp + 1) * 1024)
                nc.gpsimd.dma_start(conv_sb[:, :, sl], conv_r[:, :, sl])

            # ---- persistent per-batch tensors ------------------------------
            qT_sb = data.tile([QROWS, HW], qdt)
            kT_sb = data.tile([QROWS, N], qdt)
            vT_sb = data.tile([d, N], bf16)
            vitT_sb = data.tile([128, 6, N], bf16)
            v_sb = data.tile([128, CH, 65], bf16)   # V' = [V, ones]
            kqb_sb = const.tile([128, 2 * CH], f32)  # (q_b . K)/8 per key (x2 cols)
            ob2_sb = const.tile([128, 2], f32)      # out_b + out_w @ v_b
            vb_bf = const.tile([d, 1], bf16)
            qb2 = const.tile([d, 2], qdt)
            nc.vector.memset(v_sb[:, :, 64:65], 1.0)
            nc.vector.tensor_copy(vb_bf, vb_sb)
            nc.vector.tensor_copy(qb2[:, 0:1], qb_sb)
            nc.vector.tensor_copy(qb2[:, 1:2], qb_sb)

            wqT = const.tile([128, 2, d], qdt)
            wkT = const.tile([128, 6, d], bf16)
            wvT = const.tile([128, 6, d], bf16)
            woT = const.tile([d, 2, 128], bf16)

            out_r = out.rearrange("(t p) f -> p t f", p=128)

            # Q projection per 512-query block (f32r: no input cast)
            def q_proj(j):
                sl = slice(j * GQ, (j + 1) * GQ)
                qp = pqp.tile([d, GQ], f32, tag="qp")
                for t in range(2):
                    nc.tensor.matmul(
                        qp,
                        wqT[:, t, :],
                        conv_sb[:, t, sl],
                        start=(t == 0), stop=(t == 1),
                    )
                nc.vector.tensor_copy(qT_sb[0:d, sl], qp)
                if PACK_S:
                    nc.sync.dma_start(qT_sb[64:128, sl], qT_sb[0:64, sl])

            # ================= head phase ==================================
            with (
                tc.tile_pool(name="ptr", bufs=2, space="PSUM") as ptr,
                tc.tile_pool(name="phead", bufs=2, space="PSUM") as phead,
            ):
                # -- weight transposes (fp32 transpose mode) -----------------
                for t in range(2):
                    ps = ptr.tile([128, 3, 128], f32, tag="tr")
                    nc.tensor.transpose(
                        ps[:, 0, 0:d], wq_raw[:, t * 128 : (t + 1) * 128],
                        idf[0:d, 0:d],
                    )
                    nc.vector.tensor_copy(wqT[:, t, :], ps[:, 0, 0:d])
                for c6 in range(6):
                    ps = ptr.tile([128, 3, 128], f32, tag="tr")
                    nc.tensor.transpose(
                        ps[:, 0, 0:d], wk_raw[:, c6 * 128 : (c6 + 1) * 128],
                        idf[0:d, 0:d],
                    )
                    nc.tensor.transpose(
                        ps[:, 1, 0:d], wv_raw[:, c6 * 128 : (c6 + 1) * 128],
                        idf[0:d, 0:d],
                    )
                    nc.vector.tensor_copy(wkT[:, c6, :], ps[:, 0, 0:d])
                    nc.vector.tensor_copy(wvT[:, c6, :], ps[:, 1, 0:d])
                for t in range(2):
                    ps = ptr.tile([128, 3, 128], f32, tag="tr")
                    nc.tensor.transpose(ps[0:d, 0, :], wo_raw[:, t, :], idf)
                    nc.vector.tensor_copy(woT[:, t, :], ps[0:d, 0, :])

                # early Q projections fill the PE while vit streams in
                q_proj(0)
                q_proj(1)

                def vit_transpose(nch, eng):
                    for dg in range(2):
                        pst = ptr.tile([128, 3, 128], f32, tag="tr")
                        for k3 in range(3):
                            dch = dg * 3 + k3
                            nc.tensor.matmul(
                                pst[:, k3, :],
                                vit_sb[:, nch, dch * 128 : (dch + 1) * 128],
                                idb,
                                start=True, stop=True,
                            )
                        dst = vitT_sb[:, dg * 3 : (dg + 1) * 3,
                                      nch * 128 : (nch + 1) * 128]
                        if eng == "v":
                            nc.vector.tensor_copy(dst, pst)
                        else:
                            nc.scalar.activation(dst, pst, func=Copy)

                def kv_proj(h):
                    sl = slice(h * 512, (h + 1) * 512)
                    kp = phead.tile([d, 512], f32, tag="kv")
                    for c6 in range(6):
                        nc.tensor.matmul(
                            kp, wkT[:, c6, :], vitT_sb[:, c6, sl],
                            start=(c6 == 0), stop=(c6 == 5),
                        )
                    nc.scalar.activation(kT_sb[0:d, sl], kp, func=Copy)
                    vp = phead.tile([d, 512], f32, tag="kv")
                    for c6 in range(6):
                        nc.tensor.matmul(
                            vp, wvT[:, c6, :], vitT_sb[:, c6, sl],
                            start=(c6 == 0), stop=(c6 == 5),
                        )
                    nc.scalar.activation(vT_sb[:, sl], vp, func=Copy)

                def kqb_mms(clo, chi):
                    kqp = phead.tile([128, 16], f32, tag="misc")
                    for c in range(clo, chi):
                        i = c - clo
                        nc.tensor.matmul(
                            kqp[:, 2 * i : 2 * i + 2],
                            kT_sb[0:d, c * 128 : (c + 1) * 128],
                            qb2,
                            start=True, stop=True,
                        )
                    nc.vector.tensor_scalar_mul(
                        kqb_sb[:, 2 * clo : 2 * chi],
                        kqp[:, 0 : 2 * (chi - clo)], 0.125
                    )

                # first vit half -> kT/vT for keys 0-511
                for nch in range(4):
                    vit_transpose(nch, "v" if nch % 2 == 0 else "s")
                kv_proj(0)
                kqb_mms(0, 4)
                if PACK_S:
                    nc.sync.dma_start(kT_sb[64:128, 0:512], kT_sb[0:64, 0:512])

                # second vit half
                for nch in range(4, 8):
                    vit_transpose(nch, "v" if nch % 2 == 0 else "s")
                kv_proj(1)
                kqb_mms(4, 8)
                if PACK_S:
                    nc.sync.dma_start(
                        kT_sb[64:128, 512:1024], kT_sb[0:64, 512:1024]
                    )

                # ob2 = out_b + out_w @ v_b
                obp = phead.tile([128, 8], f32, tag="misc")
                for t in range(2):
                    nc.tensor.matmul(
                        obp[:, t : t + 1], woT[:, t, :], vb_bf,
                        start=True, stop=True,
                    )
                nc.vector.tensor_tensor(ob2_sb, obp[:, 0:2], ob_sb, add)

                # V = transpose(V^T) via identity matmuls
                for c in range(CH):
                    pst = ptr.tile([128, 3, 128], f32, tag="tr")
                    nc.tensor.matmul(
                        pst[:, 0, 0:d],
                        vT_sb[:, c * 128 : (c + 1) * 128],
                        idb[0:d, 0:d],
                        start=True, stop=True,
                    )
                    nc.vector.tensor_copy(v_sb[:, c, 0:d], pst[:, 0, 0:d])

            if phase == "head":
                dumm = data.tile([128, 2, GQ], f32)
                nc.vector.memset(dumm, 0.0)
                nc.sync.dma_start(out_r[:, :, 0:GQ], dumm)

            # ================= steady state ================================
            with (
                tc.tile_pool(name="spool", bufs=2, space="PSUM") as spool,
                tc.tile_pool(name="opsum", bufs=2, space="PSUM") as opsum,
                tc.tile_pool(name="fpool", bufs=2, space="PSUM") as fpool,
            ):
                def s_exp(c, g, e_tile):
                    base = 64 * (c % 2) if PACK_S else 0
                    sl = slice(g * GQ, (g + 1) * GQ)
                    sp = spool.tile([128, GQ], f32, tag="s")
                    nc.tensor.matmul(
                        sp,
                        kT_sb[base : base + d,
                              c * 128 : (c + 1) * 128],
                        qT_sb[base : base + d, sl],
                        start=True, stop=True,
                    )
                    nc.scalar.activation(
                        e_tile[:, c, :], sp, func=Exp,
                        bias=kqb_sb[:, 2 * c : 2 * c + 1], scale=0.125,
                    )

                def make_tail(g, o_ps):
                    """Normalize + out-project + store block g; steps are
                    scattered into the next block's c-slots."""
                    r0_sb = work.tile([1, GQ], f32, tag="r0")
                    r_sb = work.tile([1, GQ], f32, tag="r")
                    rb_sb = work.tile([64, GQ], f32, tag="rb")
                    ot = work.tile([d, GQ], bf16, tag="ot")
                    out_sb = opool.tile([128, 2, GQ], f32, tag="out")
                    fps = []

                    def t_recip():
                        if taps_on and g in (0, 7):
                            oc = data.tile([65, GQ], f32, name=f"tap_o{g}")
                            nc.vector.tensor_copy(oc, o_ps)
                            nc.gpsimd.dma_start(
                                _DBG_PARAMS[f"dbg_o{g}"][:, :], oc
                            )
                        nc.vector.tensor_copy(r0_sb, o_ps[64:65, :])
                        nc.vector.reciprocal_approx_fast(r_sb, r0_sb)

                    def t_bcast():
                        nc.gpsimd.partition_broadcast(rb_sb, r_sb)

                    def t_mult():
                        nc.vector.tensor_tensor(ot, o_ps[0:d, :], rb_sb, mult)

                    def t_proj(t):
                        def f():
                            fp = fpool.tile([128, GQ], f32, tag="f")
                            fps.append(fp)
                            nc.tensor.matmul(
                                fp, woT[0:d, t, :], ot, start=True, stop=True
                            )
                        return f

                    def t_add(t):
                        def f():
                            nc.vector.tensor_scalar_add(
                                out_sb[:, t, :], fps[t], ob2_sb[:, t : t + 1]
                            )
                        return f

                    def t_store():
                        if taps_on and g == 7:
                            nc.gpsimd.dma_start(_DBG_PARAMS["dbg_r7"][:, :], r_sb)
                            nc.gpsimd.dma_start(_DBG_PARAMS["dbg_rb7"][:, :], rb_sb)
                            otc = data.tile([64, GQ], f32, name="tap_ot7")
                            nc.vector.tensor_copy(otc, ot)
                            nc.gpsimd.dma_start(_DBG_PARAMS["dbg_ot7"][:, :], otc)
                            nc.gpsimd.dma_start(
                                _DBG_PARAMS["dbg_osb7"][:, :, :], out_sb
                            )
                        nc.sync.dma_start(
                            out_r[:, :, g * GQ : (g + 1) * GQ], out_sb
                        )

                    return [t_recip, t_bcast, t_mult, t_proj(0), t_add(0),
                            t_proj(1), t_add(1), t_store]

                # prologue: S/Exp for block 0 (alternating row groups)
                e_tiles = {0: epool.tile([128, CH, GQ], bf16, tag="e", name="e0")}
                for c in range(CH if phase != "head" else 0):
                    s_exp(c, 0, e_tiles[0])
                q_proj(2)
                q_proj(3)
                if taps_on:
                    e032 = data.tile([128, CH, GQ], f32)
                    nc.vector.tensor_copy(e032, e_tiles[0])
                    nc.gpsimd.dma_start(_DBG_PARAMS["dbg_e0"][...], e032)
                if phase == "prologue":
                    dumm = data.tile([128, 2, GQ], f32)
                    nc.vector.memset(dumm, 0.0)
                    nc.sync.dma_start(out_r[:, :, 0:GQ], dumm)

                qproj_sched = {0: [4, 5], 1: [6, 7]}

                tail = []
                for g in range(NG if phase in ("full", "oloop") else 0):
                    e_cur = e_tiles.pop(g)
                    o_ps = opsum.tile([65, GQ], f32, tag="o")
                    if g + 1 < NG:
                        e_tiles[g + 1] = epool.tile(
                            [128, CH, GQ], bf16, tag="e", name=f"e{g + 1}"
                        )
                    for c in range(CH):
                        if SPLIT_O:
                            # O in two K=64 halves on different PE row
                            # groups: adjacent matmuls execute concurrently.
                            nc.tensor.matmul(
                                o_ps, v_sb[0:64, c, :], e_cur[0:64, c, :],
                                start=(c == 0), stop=False,
                            )
                            if g + 1 < NG:
                                s_exp(c, g + 1, e_tiles[g + 1])
                            nc.tensor.matmul(
                                o_ps, v_sb[64:128, c, :], e_cur[64:128, c, :],
                                start=False, stop=(c == CH - 1),
                            )
                        else:
                            nc.tensor.matmul(
                                o_ps, v_sb[:, c, :], e_cur[:, c, :],
                                start=(c == 0), stop=(c == CH - 1),
                            )
                            if g + 1 < NG:
                                s_exp(c, g + 1, e_tiles[g + 1])
                        if c < len(tail):
                            tail[c]()
                    for j in qproj_sched.get(g, []):
                        q_proj(j)
                    if phase == "full":
                        tail = make_tail(g, o_ps)
                if taps_on:
                    import concourse.mybir as _mb
                    def _tap(name, ap):
                        t32 = data.tile(list(ap.shape), f32, name="tap_" + name)
                        nc.vector.tensor_copy(t32, ap)
                        nc.gpsimd.dma_start(_DBG_PARAMS[name][...], t32)
                    _tap("dbg_wqT", wqT)
                    _tap("dbg_wkT", wkT)
                    _tap("dbg_woT", woT)
                    _tap("dbg_qT", qT_sb[0:64, :])
                    _tap("dbg_kT", kT_sb[0:64, :])
                    _tap("dbg_vT", vT_sb)
                    _tap("dbg_v", v_sb)
                    _tap("dbg_kqb", kqb_sb)
                    _tap("dbg_ob2", ob2_sb)
                    _tap("dbg_vitT", vitT_sb)
                if phase == "full":
                    for step in tail:
                        step()
                else:
                    dumm2 = data.tile([128, 2, GQ], f32)
                    nc.vector.memset(dumm2, 0.0)
                    nc.sync.dma_start(out_r[:, :, 0:GQ], dumm2)

    nc.finalize()
    return nc


def _get_nc():
    global _CACHED_NC
    if _CACHED_NC is None:
        _CACHED_NC = _build_nc()
    return _CACHED_NC


def make_in_maps(inputs):
    conv_feat = np.asarray(inputs["conv_feat"], dtype=np.float32)
    vit_feat = np.asarray(inputs["vit_feat"], dtype=np.float32)
    weights = {
        name: np.ascontiguousarray(np.asarray(inputs[name], dtype=np.float32))
        for name in ("q_w", "q_b", "k_w", "v_w", "v_b", "out_w", "out_b")
    }
    in_maps = []
    for b in range(B):
        m = dict(weights)
        m["conv_feat"] = np.ascontiguousarray(conv_feat[b].reshape(C, HW))
        m["vit_feat"] = np.ascontiguousarray(vit_feat[b])
        in_maps.append(m)
    return in_maps


def kernel(**inputs) -> np.ndarray:
    from concourse.bass_utils import run_bass_kernel_spmd

    nc = _get_nc()
    res = run_bass_kernel_spmd(nc, make_in_maps(inputs), list(range(B)))
    return np.stack(
        [res.results[b]["out"].reshape(C, H, W) for b in range(B)]
    ).astype(np.float32)


# revision 19
# speedup vs baseline: 1.1647x; 1.1647x over previous
"""Trainium2 Bass kernel for CrossAttention2d.

Reference computation (per batch b):
    q = conv_feat[b] (as [C, HW]) projected -> [HW, d], + q_b
    k, v = vit_feat[b] [N, D] projected -> [N, d], + biases
    attn = softmax(q @ k.T / sqrt(d))          [HW, N]
    o = attn @ v                               [HW, d]
    out = o @ out_w.T + out_b -> [C, HW]

Sharding: data-parallel over batch B=8 across the 8 NeuronCores; each core
computes one full batch element.

Bias folding (exact):
  - k_b: (q+q_b).k_b is constant along the key axis -> cancels in softmax;
    k_b is never loaded.
  - q_b: logit = q_raw.K + (q_b.K); the latter depends only on the key, so
    it becomes the per-partition bias of the Exp activation:
    E = exp(S*0.125 + kqb*0.125), kqb[n] = sum_d K^T[d,n] q_b[d].
  - v_b: attn rows sum to 1 so attn@(V+vb) = attn@V + vb; folded into the
    output bias ob2 = out_b + out_w @ v_b.

Engine/DMA plan (per core), all three DMA queues in parallel:
  sync (SP HWDGE)    : weights, conv (f32r bytes, no cast), output stores
  scalar (ACT HWDGE) : vit chunks 0,1,4,5 as f32r (bitcast, no cast)
  gpsimd (SWDGE)     : vit chunks 2,3,6,7 with fp32->bf16 cast

  PE  : Q projection in f32r straight off the fp32 conv bytes (f32r moving
        streams at 2 cyc/col on HW - acceptable for 16 matmuls, and it
        avoids an 8.6us ACT cast of conv). Everything hot (S, O, out-proj)
        in bf16 at 1 cyc/col with fast weight load. vit transposed via
        identity matmuls (counts toward PE HAM warmth).
  ACT : Exp at [128, 1024] granularity (amortizes ~0.2us/instr overhead),
        plus a few head copies.
  DVE : PSUM->SBUF copies, reciprocal, normalize multiply, bias add.

V'' = [V | ones-block]: columns 64..127 of the O' stationary are all-ones,
so O' rows 64..127 all hold the softmax denominator; the reciprocal runs
as a full 64-partition DVE op. (Single-partition [1,N] DVE ops measure ~3x
slower, and reciprocal_approx_fast reading PSUM directly returns garbage
on HW.)

Steady state: 8 query blocks of 512 for O/normalize/store; S/Exp runs at
1024-wide blocks on even iterations; the late Q projections borrow spool
tiles inside the loop. Block g's O accumulation interleaves with the next
S/Exp block, and block g-1's tail is scattered into g's c-slots.
"""

import numpy as np

B = 8
C = 256
H = W = 64
HW = 4096
N = 1024
D = 768
d = 64
GQ = 512           # O / tail block width
NG = HW // GQ      # 8
GS = 1024          # S / Exp block width
NS = HW // GS      # 4
CH = N // 128      # 8 key chunks

_CACHED_NC = None
_DBG_PARAMS = {}

# vit chunk -> (tile kind, slot): "f" = f32r via scalar HWDGE,
# "c" = bf16 via gpsimd SWDGE cast
_VIT_MAP = {0: ("f", 0), 1: ("f", 1), 2: ("c", 0), 3: ("c", 1),
            4: ("f", 2), 5: ("f", 3), 6: ("c", 2), 7: ("c", 3)}


def _build_nc():
    import os

    import concourse.mybir as mybir
    from concourse import bacc
    from concourse.masks import make_identity
    from concourse.tile import TileContext

    dt = mybir.dt
    f32 = dt.float32
    f32r = dt.float32r
    bf16 = dt.bfloat16
    Exp = mybir.ActivationFunctionType.Exp
    Copy = mybir.ActivationFunctionType.Copy
    mult = mybir.AluOpType.mult
    add = mybir.AluOpType.add

    taps_on = os.environ.get("BASS_DEBUG_TAPS") == "1"

    nc = bacc.Bacc(None)

    conv = nc.declare_dram_parameter("conv_feat", [C, HW], f32r, isOutput=False)
    vit = nc.declare_dram_parameter("vit_feat", [N, D], f32, isOutput=False)
    q_w = nc.declare_dram_parameter("q_w", [d, C], f32, isOutput=False)
    q_b = nc.declare_dram_parameter("q_b", [d], f32, isOutput=False)
    k_w = nc.declare_dram_parameter("k_w", [d, D], f32, isOutput=False)
    v_w = nc.declare_dram_parameter("v_w", [d, D], f32, isOutput=False)
    v_b = nc.declare_dram_parameter("v_b", [d], f32, isOutput=False)
    out_w = nc.declare_dram_parameter("out_w", [C, d], f32, isOutput=False)
    out_b = nc.declare_dram_parameter("out_b", [C], f32, isOutput=False)
    out = nc.declare_dram_parameter("out", [C, HW], f32, isOutput=True)

    if taps_on:
        for tname, tshape in [
            ("dbg_qT", [64, HW]), ("dbg_kT", [64, N]), ("dbg_vT", [d, N]),
            ("dbg_v", [128, CH, 128]), ("dbg_kqb", [128, 16]),
            ("dbg_ob2", [128, 2]), ("dbg_e0", [128, CH, GS]),
        ]:
            _DBG_PARAMS[tname] = nc.declare_dram_parameter(
                tname, tshape, f32, isOutput=True
            )

    with TileContext(nc) as tc:
        with (
            tc.tile_pool(name="const", bufs=1) as const,
            tc.tile_pool(name="data", bufs=1) as data,
            tc.tile_pool(name="epool", bufs=2) as epool,
            tc.tile_pool(name="opool", bufs=2) as opool,
            tc.tile_pool(name="work", bufs=2) as work,
        ):
            # ---- identities ------------------------------------------------
            idf = const.tile([128, 128], f32)
            make_identity(nc, idf)
            idb = const.tile([128, 128], bf16)
            make_identity(nc, idb)
            idr = const.tile([128, 128], f32r)
            nc.vector.tensor_copy(idr, idf)

            # ---- weight loads: sync queue, fp32 ---------------------------
            wq_raw = const.tile([d, C], f32)
            nc.sync.dma_start(wq_raw, q_w[:, :])
            wk_raw = const.tile([d, D], f32)
            nc.sync.dma_start(wk_raw, k_w[:, :])
            wv_raw = const.tile([d, D], f32)
            nc.sync.dma_start(wv_raw, v_w[:, :])
            wo_raw = const.tile([128, 2, d], f32)
            nc.sync.dma_start(wo_raw, out_w.rearrange("(t p) e -> p t e", p=128))
            qb_sb = const.tile([d, 1], f32)
            nc.sync.dma_start(qb_sb, q_b.rearrange("(a b) -> a b", b=1))
            vb_sb = const.tile([d, 1], f32)
            nc.sync.dma_start(vb_sb, v_b.rearrange("(a b) -> a b", b=1))
            ob_sb = const.tile([128, 2], f32)
            nc.sync.dma_start(ob_sb, out_b.rearrange("(t p) -> p t", p=128))

            # ---- input loads ----------------------------------------------
            vit_r = vit.rearrange("(c p) e -> p c e", p=128)
            # f32r chunks 0,1 / 4,5 on the scalar HWDGE queue (bitcast)
            vit_f = data.tile([128, 4, D], f32r)
            nc.scalar.dma_start(vit_f[:, 0:2, :], vit_r[:, 0:2, :].bitcast(f32r))
            # bf16 chunks 2,3 / 6,7 on the gpsimd SWDGE queue (cast)
            vit_c = data.tile([128, 4, D], bf16)
            nc.gpsimd.dma_start(vit_c[:, 0:2, :], vit_r[:, 2:4, :])
            nc.scalar.dma_start(vit_f[:, 2:4, :], vit_r[:, 4:6, :].bitcast(f32r))
            nc.gpsimd.dma_start(vit_c[:, 2:4, :], vit_r[:, 6:8, :])
            # conv: sync HWDGE, raw f32 bytes consumed as f32r
            conv_sb = data.tile([128, 2, HW], f32r)
            conv_r = conv.rearrange("(t p) f -> p t f", p=128)
            for jp in range(4):
                sl = slice(jp * 1024, (jp + 1) * 1024)
                nc.sync.dma_start(conv_sb[:, :, sl], conv_r[:, :, sl])

            # ---- persistent per-batch tensors ------------------------------
            qT_sb = data.tile([d, HW], bf16)
            kT_sb = data.tile([d, N], bf16)
            vT_sb = data.tile([d, N], bf16)
            vitT_sb = data.tile([128, 6, N], bf16)
            v_sb = data.tile([128, CH, 128], bf16)  # V'' = [V | ones-block]
            kqb_sb = const.tile([128, CH], f32)     # (q_b . K)/8 per key
            ob2_sb = const.tile([128, 2], f32)      # out_b + out_w @ v_b
            vb_bf = const.tile([d, 1], bf16)
            qb2 = const.tile([d, 1], bf16)
            nc.vector.memset(v_sb[:, :, 64:128], 1.0)
            nc.vector.tensor_copy(vb_bf, vb_sb)
            nc.vector.tensor_copy(qb2, qb_sb)

            wqT = const.tile([128, 2, d], f32r)
            wkT = const.tile([128, 6, d], bf16)
            wvT = const.tile([128, 6, d], bf16)
            woT = const.tile([d, 2, 128], bf16)

            out_r = out.rearrange("(t p) f -> p t f", p=128)

            # ================= head phase ==================================
            with (
                tc.tile_pool(name="ptr", bufs=2, space="PSUM") as ptr,
                tc.tile_pool(name="phead", bufs=2, space="PSUM") as phead,
            ):
                # -- weight transposes (fp32 transpose mode) -----------------
                for t in range(2):
                    ps = ptr.tile([128, 3, 128], f32, tag="tr")
                    nc.tensor.transpose(
                        ps[:, 0, 0:d], wq_raw[:, t * 128 : (t + 1) * 128],
                        idf[0:d, 0:d],
                    )
                    nc.vector.tensor_copy(wqT[:, t, :], ps[:, 0, 0:d])
                for c6 in range(6):
                    ps = ptr.tile([128, 3, 128], f32, tag="tr")
                    nc.tensor.transpose(
                        ps[:, 0, 0:d], wk_raw[:, c6 * 128 : (c6 + 1) * 128],
                        idf[0:d, 0:d],
                    )
                    nc.tensor.transpose(
                        ps[:, 1, 0:d], wv_raw[:, c6 * 128 : (c6 + 1) * 128],
                        idf[0:d, 0:d],
                    )
                    nc.vector.tensor_copy(wkT[:, c6, :], ps[:, 0, 0:d])
                    nc.vector.tensor_copy(wvT[:, c6, :], ps[:, 1, 0:d])
                for t in range(2):
                    ps = ptr.tile([128, 3, 128], f32, tag="tr")
                    nc.tensor.transpose(ps[0:d, 0, :], wo_raw[:, t, :], idf)
                    nc.vector.tensor_copy(woT[:, t, :], ps[0:d, 0, :])

                def vit_transpose(nch, eng):
                    kind, slot = _VIT_MAP[nch]
                    src = vit_f if kind == "f" else vit_c
                    ident = idr if kind == "f" else idb
                    for dg in range(2):
                        pst = ptr.tile([128, 3, 128], f32, tag="tr")
                        for k3 in range(3):
                            dch = dg * 3 + k3
                            nc.tensor.matmul(
                                pst[:, k3, :],
                                src[:, slot, dch * 128 : (dch + 1) * 128],
                                ident,
                                start=True, stop=True,
                            )
                        dst = vitT_sb[:, dg * 3 : (dg + 1) * 3,
                                      nch * 128 : (nch + 1) * 128]
                        if eng == "v":
                            nc.vector.tensor_copy(dst, pst)
                        else:
                            nc.scalar.activation(dst, pst, func=Copy)

                def kv_proj(h):
                    sl = slice(h * 512, (h + 1) * 512)
                    kp = phead.tile([d, 512], f32, tag="kv")
                    for c6 in range(6):
                        nc.tensor.matmul(
                            kp, wkT[:, c6, :], vitT_sb[:, c6, sl],
                            start=(c6 == 0), stop=(c6 == 5),
                        )
                    nc.scalar.activation(kT_sb[:, sl], kp, func=Copy)
                    vp = phead.tile([d, 512], f32, tag="kv")
                    for c6 in range(6):
                        nc.tensor.matmul(
                            vp, wvT[:, c6, :], vitT_sb[:, c6, sl],
                            start=(c6 == 0), stop=(c6 == 5),
                        )
                    nc.scalar.activation(vT_sb[:, sl], vp, func=Copy)

                def kqb_mms(clo, chi):
                    kqp = phead.tile([128, 16], f32, tag="misc")
                    for c in range(clo, chi):
                        i = c - clo
                        nc.tensor.matmul(
                            kqp[:, i : i + 1],
                            kT_sb[:, c * 128 : (c + 1) * 128],
                            qb2,
                            start=True, stop=True,
                        )
                    nc.vector.tensor_scalar_mul(
                        kqb_sb[:, clo:chi], kqp[:, 0 : chi - clo], 0.125
                    )

                def q_proj_head(j):
                    sl = slice(j * GQ, (j + 1) * GQ)
                    qp = phead.tile([d, GQ], f32, tag="qp")
                    for t in range(2):
                        nc.tensor.matmul(
                            qp, wqT[:, t, :], conv_sb[:, t, sl],
                            start=(t == 0), stop=(t == 1),
                        )
                    nc.vector.tensor_copy(qT_sb[:, sl], qp)

                # first vit n-half -> kT/vT for keys 0-511
                for nch in range(4):
                    vit_transpose(nch, "v" if nch % 2 == 0 else "s")
                kv_proj(0)
                kqb_mms(0, 4)
                for j in range(4):
                    q_proj_head(j)

                # second vit n-half
                for nch in range(4, 8):
                    vit_transpose(nch, "v" if nch % 2 == 0 else "s")
                kv_proj(1)
                kqb_mms(4, 8)

                # ob2 = out_b + out_w @ v_b
                obp = phead.tile([128, 16], f32, tag="misc")
                for t in range(2):
                    nc.tensor.matmul(
                        obp[:, t : t + 1], woT[:, t, :], vb_bf,
                        start=True, stop=True,
                    )
                nc.vector.tensor_tensor(ob2_sb, obp[:, 0:2], ob_sb, add)

                # V = transpose(V^T) via identity matmuls
                for c in range(CH):
                    pst = ptr.tile([128, 3, 128], f32, tag="tr")
                    nc.tensor.matmul(
                        pst[:, 0, 0:d],
                        vT_sb[:, c * 128 : (c + 1) * 128],
                        idb[0:d, 0:d],
                        start=True, stop=True,
                    )
                    nc.vector.tensor_copy(v_sb[:, c, 0:d], pst[:, 0, 0:d])

            # ================= steady state ================================
            with (
                tc.tile_pool(name="spool", bufs=2, space="PSUM") as spool,
                tc.tile_pool(name="opsum", bufs=2, space="PSUM") as opsum,
                tc.tile_pool(name="fpool", bufs=2, space="PSUM") as fpool,
            ):
                def s_exp(c, Gs, e_tile):
                    sp = spool.tile([128, GS], f32, tag="s")
                    for h2 in range(2):
                        sl = slice(Gs * GS + h2 * 512, Gs * GS + h2 * 512 + 512)
                        nc.tensor.matmul(
                            sp[:, h2 * 512 : (h2 + 1) * 512],
                            kT_sb[:, c * 128 : (c + 1) * 128],
                            qT_sb[:, sl],
                            start=True, stop=True,
                        )
                    nc.scalar.activation(
                        e_tile[:, c, :], sp, func=Exp,
                        bias=kqb_sb[:, c : c + 1], scale=0.125,
                    )

                def q_proj_steady(j):
                    sl = slice(j * GQ, (j + 1) * GQ)
                    qpt = spool.tile([128, GS], f32, tag="s", name=f"qp{j}")
                    qp = qpt[0:d, 0:GQ]
                    for t in range(2):
                        nc.tensor.matmul(
                            qp, wqT[:, t, :], conv_sb[:, t, sl],
                            start=(t == 0), stop=(t == 1),
                        )
                    nc.vector.tensor_copy(qT_sb[:, sl], qp)

                def make_tail(g, o_ps):
                    """Normalize + out-project + store block g; steps are
                    scattered into the next block's c-slots."""
                    o64 = work.tile([64, GQ], f32, tag="o64")
                    rb_sb = work.tile([64, GQ], f32, tag="rb")
                    ot = work.tile([d, GQ], bf16, tag="ot")
                    out_sb = opool.tile([128, 2, GQ], f32, tag="out")
                    fps = []

                    def t_copy():
                        nc.vector.tensor_copy(o64, o_ps[64:128, :])

                    def t_recip():
                        nc.vector.reciprocal_approx_fast(rb_sb, o64)

                    def t_mult():
                        nc.vector.tensor_tensor(ot, o_ps[0:d, :], rb_sb, mult)

                    def t_proj(t):
                        def f():
                            fp = fpool.tile([128, GQ], f32, tag="f")
                            fps.append(fp)
                            nc.tensor.matmul(
                                fp, woT[0:d, t, :], ot, start=True, stop=True
                            )
                        return f

                    def t_add(t):
                        def f():
                            nc.vector.tensor_scalar_add(
                                out_sb[:, t, :], fps[t], ob2_sb[:, t : t + 1]
                            )
                        return f

                    def t_store():
                        nc.sync.dma_start(
                            out_r[:, :, g * GQ : (g + 1) * GQ], out_sb
                        )

                    return [t_copy, t_recip, t_mult, t_proj(0), t_add(0),
                            t_proj(1), t_add(1), t_store]

                # prologue: S/Exp for the first 1024-query block
                e_tiles = {0: epool.tile([128, CH, GS], bf16, tag="e", name="e0")}
                for c in range(CH):
                    s_exp(c, 0, e_tiles[0])

                if taps_on:
                    e032 = data.tile([128, CH, GS], f32)
                    nc.vector.tensor_copy(e032, e_tiles[0])
                    nc.gpsimd.dma_start(_DBG_PARAMS["dbg_e0"][:, :, :], e032)

                qproj_sched = {0: [4, 5], 1: [6, 7]}
                tail = []
                for g in range(NG):
                    Gs = g // 2
                    e_cur = e_tiles[Gs]
                    o_ps = opsum.tile([128, GQ], f32, tag="o")
                    half = slice((g % 2) * 512, (g % 2) * 512 + 512)
                    emit_s = (g % 2 == 0) and (Gs + 1 < NS)
                    if emit_s:
                        e_tiles[Gs + 1] = epool.tile(
                            [128, CH, GS], bf16, tag="e", name=f"e{Gs + 1}"
                        )
                    for c in range(CH):
                        nc.tensor.matmul(
                            o_ps, v_sb[:, c, :], e_cur[:, c, half],
                            start=(c == 0), stop=(c == CH - 1),
                        )
                        if emit_s:
                            s_exp(c, Gs + 1, e_tiles[Gs + 1])
                        if c < len(tail):
                            tail[c]()
                    if g % 2 == 1:
                        e_tiles.pop(Gs)
                    for j in qproj_sched.get(g, []):
                        q_proj_steady(j)
                    tail = make_tail(g, o_ps)
                for step in tail:
                    step()

                if taps_on:
                    def _tap(name, ap):
                        t32 = data.tile(list(ap.shape), f32, name="tap_" + name)
                        nc.vector.tensor_copy(t32, ap)
                        args = tuple(slice(None) for _ in ap.shape)
                        nc.gpsimd.dma_start(_DBG_PARAMS[name][args], t32)
                    _tap("dbg_qT", qT_sb)
                    _tap("dbg_kT", kT_sb)
                    _tap("dbg_vT", vT_sb)
                    _tap("dbg_v", v_sb)
                    _tap("dbg_kqb", kqb_sb)
                    _tap("dbg_ob2", ob2_sb)

    nc.finalize()
    return nc


def _get_nc():
    global _CACHED_NC
    if _CACHED_NC is None:
        _CACHED_NC = _build_nc()
    return _CACHED_NC


def make_in_maps(inputs):
    conv_feat = np.asarray(inputs["conv_feat"], dtype=np.float32)
    vit_feat = np.asarray(inputs["vit_feat"], dtype=np.float32)
    weights = {
        name: np.ascontiguousarray(np.asarray(inputs[name], dtype=np.float32))
        for name in ("q_w", "q_b", "k_w", "v_w", "v_b", "out_w", "out_b")
    }
    in_maps = []
    for b in range(B):
        m = dict(weights)
        m["conv_feat"] = np.ascontiguousarray(conv_feat[b].reshape(C, HW))
        m["vit_feat"] = np.ascontiguousarray(vit_feat[b])
        in_maps.append(m)
    return in_maps


def kernel(**inputs) -> np.ndarray:
    from concourse.bass_utils import run_bass_kernel_spmd

    nc = _get_nc()
    res = run_bass_kernel_spmd(nc, make_in_maps(inputs), list(range(B)))
    return np.stack(
        [res.results[b]["out"].reshape(C, H, W) for b in range(B)]
    ).astype(np.float32)


# revision 20
# speedup vs baseline: 1.1664x; 1.0014x over previous
"""Trainium2 Bass kernel for CrossAttention2d.

Reference computation (per batch b):
    q = conv_feat[b] (as [C, HW]) projected -> [HW, d], + q_b
    k, v = vit_feat[b] [N, D] projected -> [N, d], + biases
    attn = softmax(q @ k.T / sqrt(d))          [HW, N]
    o = attn @ v                               [HW, d]
    out = o @ out_w.T + out_b -> [C, HW]

Sharding: data-parallel over batch B=8 across the 8 NeuronCores; each core
computes one full batch element.

Bias folding (exact):
  - k_b: (q+q_b).k_b is constant along the key axis -> cancels in softmax;
    k_b is never loaded.
  - q_b: logit = q_raw.K + (q_b.K); the latter depends only on the key, so
    it becomes the per-partition bias of the Exp activation:
    E = exp(S*0.125 + kqb*0.125), kqb[n] = sum_d K^T[d,n] q_b[d].
  - v_b: attn rows sum to 1 so attn@(V+vb) = attn@V + vb; folded into the
    output bias ob2 = out_b + out_w @ v_b.

Engine/DMA plan (per core), all three DMA queues in parallel:
  sync (SP HWDGE)    : weights, conv (f32r bytes, no cast), output stores
  scalar (ACT HWDGE) : vit chunks 0,1,4,5 as f32r (bitcast, no cast)
  gpsimd (SWDGE)     : vit chunks 2,3,6,7 with fp32->bf16 cast

  PE  : Q projection in f32r straight off the fp32 conv bytes (f32r moving
        streams at 2 cyc/col on HW - acceptable for 16 matmuls, and it
        avoids an 8.6us ACT cast of conv). Everything hot (S, O, out-proj)
        in bf16 at 1 cyc/col with fast weight load. vit transposed via
        identity matmuls (counts toward PE HAM warmth).
  ACT : Exp at [128, 1024] granularity (amortizes ~0.2us/instr overhead),
        plus a few head copies.
  DVE : PSUM->SBUF copies, reciprocal, normalize multiply, bias add.

V'' = [V | ones-block]: columns 64..127 of the O' stationary are all-ones,
so O' rows 64..127 all hold the softmax denominator; the reciprocal runs
as a full 64-partition DVE op. (Single-partition [1,N] DVE ops measure ~3x
slower, and reciprocal_approx_fast reading PSUM directly returns garbage
on HW.)

Steady state: 8 query blocks of 512 for O/normalize/store; S/Exp runs at
1024-wide blocks on even iterations; the late Q projections borrow spool
tiles inside the loop. Block g's O accumulation interleaves with the next
S/Exp block, and block g-1's tail is scattered into g's c-slots.
"""

import numpy as np

B = 8
C = 256
H = W = 64
HW = 4096
N = 1024
D = 768
d = 64
GQ = 512           # O / tail block width
NG = HW // GQ      # 8
GS = 1024          # S / Exp block width
NS = HW // GS      # 4
CH = N // 128      # 8 key chunks

_CACHED_NC = None
_DBG_PARAMS = {}

# vit chunk -> (tile kind, slot): "f" = f32r via scalar HWDGE,
# "c" = bf16 via gpsimd SWDGE cast
_VIT_MAP = {0: ("f", 0), 1: ("f", 1), 2: ("c", 0), 3: ("c", 1),
            4: ("f", 2), 5: ("f", 3), 6: ("c", 2), 7: ("c", 3)}


def _build_nc():
    import os

    import concourse.mybir as mybir
    from concourse import bacc
    from concourse.masks import make_identity
    from concourse.tile import TileContext

    dt = mybir.dt
    f32 = dt.float32
    f32r = dt.float32r
    bf16 = dt.bfloat16
    Exp = mybir.ActivationFunctionType.Exp
    Copy = mybir.ActivationFunctionType.Copy
    mult = mybir.AluOpType.mult
    add = mybir.AluOpType.add

    taps_on = os.environ.get("BASS_DEBUG_TAPS") == "1"

    nc = bacc.Bacc(None)

    conv = nc.declare_dram_parameter("conv_feat", [C, HW], f32r, isOutput=False)
    vit = nc.declare_dram_parameter("vit_feat", [N, D], f32, isOutput=False)
    q_w = nc.declare_dram_parameter("q_w", [d, C], f32, isOutput=False)
    q_b = nc.declare_dram_parameter("q_b", [d], f32, isOutput=False)
    k_w = nc.declare_dram_parameter("k_w", [d, D], f32, isOutput=False)
    v_w = nc.declare_dram_parameter("v_w", [d, D], f32, isOutput=False)
    v_b = nc.declare_dram_parameter("v_b", [d], f32, isOutput=False)
    out_w = nc.declare_dram_parameter("out_w", [C, d], f32, isOutput=False)
    out_b = nc.declare_dram_parameter("out_b", [C], f32, isOutput=False)
    out = nc.declare_dram_parameter("out", [C, HW], f32, isOutput=True)

    if taps_on:
        for tname, tshape in [
            ("dbg_qT", [64, HW]), ("dbg_kT", [64, N]), ("dbg_vT", [d, N]),
            ("dbg_v", [128, CH, 128]), ("dbg_kqb", [128, 16]),
            ("dbg_ob2", [128, 2]), ("dbg_e0", [128, CH, GS]),
        ]:
            _DBG_PARAMS[tname] = nc.declare_dram_parameter(
                tname, tshape, f32, isOutput=True
            )

    with TileContext(nc) as tc:
        with (
            tc.tile_pool(name="const", bufs=1) as const,
            tc.tile_pool(name="data", bufs=1) as data,
            tc.tile_pool(name="epool", bufs=2) as epool,
            tc.tile_pool(name="opool", bufs=2) as opool,
            tc.tile_pool(name="work", bufs=2) as work,
        ):
            # ---- identities ------------------------------------------------
            idf = const.tile([128, 128], f32)
            make_identity(nc, idf)
            idb = const.tile([128, 128], bf16)
            make_identity(nc, idb)
            idr = const.tile([128, 128], f32r)
            nc.vector.tensor_copy(idr, idf)

            # ---- weight loads: sync queue, fp32 ---------------------------
            wq_raw = const.tile([d, C], f32)
            nc.sync.dma_start(wq_raw, q_w[:, :])
            wk_raw = const.tile([d, D], f32)
            nc.sync.dma_start(wk_raw, k_w[:, :])
            wv_raw = const.tile([d, D], f32)
            nc.sync.dma_start(wv_raw, v_w[:, :])
            wo_raw = const.tile([128, 2, d], f32)
            nc.sync.dma_start(wo_raw, out_w.rearrange("(t p) e -> p t e", p=128))
            qb_sb = const.tile([d, 1], f32)
            nc.sync.dma_start(qb_sb, q_b.rearrange("(a b) -> a b", b=1))
            vb_sb = const.tile([d, 1], f32)
            nc.sync.dma_start(vb_sb, v_b.rearrange("(a b) -> a b", b=1))
            ob_sb = const.tile([128, 2], f32)
            nc.sync.dma_start(ob_sb, out_b.rearrange("(t p) -> p t", p=128))

            # ---- input loads ----------------------------------------------
            vit_r = vit.rearrange("(c p) e -> p c e", p=128)
            # f32r chunks 0,1 / 4,5 on the scalar HWDGE queue (bitcast)
            vit_f = data.tile([128, 4, D], f32r)
            nc.scalar.dma_start(vit_f[:, 0:2, :], vit_r[:, 0:2, :].bitcast(f32r))
            # bf16 chunks 2,3 / 6,7 on the gpsimd SWDGE queue (cast)
            vit_c = data.tile([128, 4, D], bf16)
            nc.gpsimd.dma_start(vit_c[:, 0:2, :], vit_r[:, 2:4, :])
            nc.scalar.dma_start(vit_f[:, 2:4, :], vit_r[:, 4:6, :].bitcast(f32r))
            nc.gpsimd.dma_start(vit_c[:, 2:4, :], vit_r[:, 6:8, :])
            # conv: sync HWDGE, raw f32 bytes consumed as f32r
            conv_sb = data.tile([128, 2, HW], f32r)
            conv_r = conv.rearrange("(t p) f -> p t f", p=128)
            for jp in range(4):
                sl = slice(jp * 1024, (jp + 1) * 1024)
                nc.sync.dma_start(conv_sb[:, :, sl], conv_r[:, :, sl])

            # ---- persistent per-batch tensors ------------------------------
            qT_sb = data.tile([d, HW], bf16)
            kT_sb = data.tile([d, N], bf16)
            vT_sb = data.tile([d, N], bf16)
            vitT_sb = data.tile([128, 6, N], bf16)
            v_sb = data.tile([128, CH, 128], bf16)  # V'' = [V | ones-block]
            kqb_sb = const.tile([128, CH], f32)     # (q_b . K)/8 per key
            ob2_sb = const.tile([128, 2], f32)      # out_b + out_w @ v_b
            vb_bf = const.tile([d, 1], bf16)
            qb2 = const.tile([d, 1], bf16)
            nc.vector.memset(v_sb[:, :, 64:128], 1.0)
            nc.vector.tensor_copy(vb_bf, vb_sb)
            nc.vector.tensor_copy(qb2, qb_sb)

            wqT = const.tile([128, 2, d], f32r)
            wkT = const.tile([128, 6, d], bf16)
            wvT = const.tile([128, 6, d], bf16)
            woT = const.tile([d, 2, 128], bf16)

            out_r = out.rearrange("(t p) f -> p t f", p=128)

            # ================= head phase ==================================
            with (
                tc.tile_pool(name="ptr", bufs=2, space="PSUM") as ptr,
                tc.tile_pool(name="phead", bufs=2, space="PSUM") as phead,
            ):
                # -- weight transposes (fp32 transpose mode) -----------------
                for t in range(2):
                    ps = ptr.tile([128, 3, 128], f32, tag="tr")
                    nc.tensor.transpose(
                        ps[:, 0, 0:d], wq_raw[:, t * 128 : (t + 1) * 128],
                        idf[0:d, 0:d],
                    )
                    nc.vector.tensor_copy(wqT[:, t, :], ps[:, 0, 0:d])
                for c6 in range(6):
                    ps = ptr.tile([128, 3, 128], f32, tag="tr")
                    nc.tensor.transpose(
                        ps[:, 0, 0:d], wk_raw[:, c6 * 128 : (c6 + 1) * 128],
                        idf[0:d, 0:d],
                    )
                    nc.tensor.transpose(
                        ps[:, 1, 0:d], wv_raw[:, c6 * 128 : (c6 + 1) * 128],
                        idf[0:d, 0:d],
                    )
                    nc.vector.tensor_copy(wkT[:, c6, :], ps[:, 0, 0:d])
                    nc.vector.tensor_copy(wvT[:, c6, :], ps[:, 1, 0:d])
                for t in range(2):
                    ps = ptr.tile([128, 3, 128], f32, tag="tr")
                    nc.tensor.transpose(ps[0:d, 0, :], wo_raw[:, t, :], idf)
                    nc.vector.tensor_copy(woT[:, t, :], ps[0:d, 0, :])

                def vit_transpose(nch, eng):
                    kind, slot = _VIT_MAP[nch]
                    src = vit_f if kind == "f" else vit_c
                    ident = idr if kind == "f" else idb
                    for dg in range(2):
                        pst = ptr.tile([128, 3, 128], f32, tag="tr")
                        for k3 in range(3):
                            dch = dg * 3 + k3
                            nc.tensor.matmul(
                                pst[:, k3, :],
                                src[:, slot, dch * 128 : (dch + 1) * 128],
                                ident,
                                start=True, stop=True,
                            )
                        dst = vitT_sb[:, dg * 3 : (dg + 1) * 3,
                                      nch * 128 : (nch + 1) * 128]
                        if eng == "v":
                            nc.vector.tensor_copy(dst, pst)
                        else:
                            nc.scalar.activation(dst, pst, func=Copy)

                def kv_proj(h):
                    sl = slice(h * 512, (h + 1) * 512)
                    kp = phead.tile([d, 512], f32, tag="kv")
                    for c6 in range(6):
                        nc.tensor.matmul(
                            kp, wkT[:, c6, :], vitT_sb[:, c6, sl],
                            start=(c6 == 0), stop=(c6 == 5),
                        )
                    nc.scalar.activation(kT_sb[:, sl], kp, func=Copy)
                    vp = phead.tile([d, 512], f32, tag="kv")
                    for c6 in range(6):
                        nc.tensor.matmul(
                            vp, wvT[:, c6, :], vitT_sb[:, c6, sl],
                            start=(c6 == 0), stop=(c6 == 5),
                        )
                    nc.scalar.activation(vT_sb[:, sl], vp, func=Copy)

                def kqb_mms(clo, chi):
                    kqp = phead.tile([128, 16], f32, tag="misc")
                    for c in range(clo, chi):
                        i = c - clo
                        nc.tensor.matmul(
                            kqp[:, i : i + 1],
                            kT_sb[:, c * 128 : (c + 1) * 128],
                            qb2,
                            start=True, stop=True,
                        )
                    nc.vector.tensor_scalar_mul(
                        kqb_sb[:, clo:chi], kqp[:, 0 : chi - clo], 0.125
                    )

                def q_proj_head(j):
                    sl = slice(j * GQ, (j + 1) * GQ)
                    qp = phead.tile([d, GQ], f32, tag="qp")
                    for t in range(2):
                        nc.tensor.matmul(
                            qp, wqT[:, t, :], conv_sb[:, t, sl],
                            start=(t == 0), stop=(t == 1),
                        )
                    nc.vector.tensor_copy(qT_sb[:, sl], qp)

                # first vit n-half -> kT/vT for keys 0-511
                for nch in range(4):
                    vit_transpose(nch, "v" if nch % 2 == 0 else "s")
                kv_proj(0)
                kqb_mms(0, 4)
                for j in range(4):
                    q_proj_head(j)
                head_done_a = True

                # second vit n-half
                for nch in range(4, 8):
                    vit_transpose(nch, "v" if nch % 2 == 0 else "s")
                kv_proj(1)
                kqb_mms(4, 8)

                # ob2 = out_b + out_w @ v_b
                obp = phead.tile([128, 16], f32, tag="misc")
                for t in range(2):
                    nc.tensor.matmul(
                        obp[:, t : t + 1], woT[:, t, :], vb_bf,
                        start=True, stop=True,
                    )
                nc.vector.tensor_tensor(ob2_sb, obp[:, 0:2], ob_sb, add)

                # V = transpose(V^T) via identity matmuls
                for c in range(CH):
                    pst = ptr.tile([128, 3, 128], f32, tag="tr")
                    nc.tensor.matmul(
                        pst[:, 0, 0:d],
                        vT_sb[:, c * 128 : (c + 1) * 128],
                        idb[0:d, 0:d],
                        start=True, stop=True,
                    )
                    nc.vector.tensor_copy(v_sb[:, c, 0:d], pst[:, 0, 0:d])

            # ================= steady state ================================
            with (
                tc.tile_pool(name="spool", bufs=3, space="PSUM") as spool,
                tc.tile_pool(name="opsum", bufs=2, space="PSUM") as opsum,
            ):
                def s_exp(c, Gs, e_tile):
                    sp = spool.tile([128, GS], f32, tag="s")
                    for h2 in range(2):
                        sl = slice(Gs * GS + h2 * 512, Gs * GS + h2 * 512 + 512)
                        nc.tensor.matmul(
                            sp[:, h2 * 512 : (h2 + 1) * 512],
                            kT_sb[:, c * 128 : (c + 1) * 128],
                            qT_sb[:, sl],
                            start=True, stop=True,
                        )
                    nc.scalar.activation(
                        e_tile[:, c, :], sp, func=Exp,
                        bias=kqb_sb[:, c : c + 1], scale=0.125,
                    )

                def q_proj_steady(j):
                    sl = slice(j * GQ, (j + 1) * GQ)
                    qpt = spool.tile([128, GS], f32, tag="s", name=f"qp{j}")
                    qp = qpt[0:d, 0:GQ]
                    for t in range(2):
                        nc.tensor.matmul(
                            qp, wqT[:, t, :], conv_sb[:, t, sl],
                            start=(t == 0), stop=(t == 1),
                        )
                    nc.vector.tensor_copy(qT_sb[:, sl], qp)

                def make_tail(g, o_ps):
                    """Normalize + out-project + store block g; steps are
                    scattered into the next block's c-slots."""
                    o64 = work.tile([64, GQ], f32, tag="o64")
                    rb_sb = work.tile([64, GQ], f32, tag="rb")
                    ot = work.tile([d, GQ], bf16, tag="ot")
                    out_sb = opool.tile([128, 2, GQ], f32, tag="out")
                    fps = []

                    def t_copy():
                        nc.vector.tensor_copy(o64, o_ps[64:128, :])

                    def t_recip():
                        nc.vector.reciprocal_approx_fast(rb_sb, o64)

                    def t_mult():
                        nc.vector.tensor_tensor(ot, o_ps[0:d, :], rb_sb, mult)

                    def t_proj(t):
                        def f():
                            fp = opsum.tile([128, GQ], f32, tag="o",
                                            name=f"fp{t}_{g}")
                            fps.append(fp)
                            nc.tensor.matmul(
                                fp, woT[0:d, t, :], ot, start=True, stop=True
                            )
                        return f

                    def t_add(t):
                        def f():
                            nc.vector.tensor_scalar_add(
                                out_sb[:, t, :], fps[t], ob2_sb[:, t : t + 1]
                            )
                        return f

                    def t_store():
                        nc.sync.dma_start(
                            out_r[:, :, g * GQ : (g + 1) * GQ], out_sb
                        )

                    return [t_copy, t_recip, t_mult, t_proj(0), t_add(0),
                            t_proj(1), t_add(1), t_store]

                # prologue: S/Exp for the first 1024-query block
                e_tiles = {0: epool.tile([128, CH, GS], bf16, tag="e", name="e0")}
                for c in range(CH):
                    s_exp(c, 0, e_tiles[0])

                if taps_on:
                    e032 = data.tile([128, CH, GS], f32)
                    nc.vector.tensor_copy(e032, e_tiles[0])
                    nc.gpsimd.dma_start(_DBG_PARAMS["dbg_e0"][:, :, :], e032)

                qproj_sched = {0: [4, 5], 1: [6, 7]}
                tail = []
                for g in range(NG):
                    Gs = g // 2
                    e_cur = e_tiles[Gs]
                    o_ps = opsum.tile([128, GQ], f32, tag="o")
                    half = slice((g % 2) * 512, (g % 2) * 512 + 512)
                    emit_s = (g % 2 == 0) and (Gs + 1 < NS)
                    if emit_s:
                        e_tiles[Gs + 1] = epool.tile(
                            [128, CH, GS], bf16, tag="e", name=f"e{Gs + 1}"
                        )
                    for c in range(CH):
                        nc.tensor.matmul(
                            o_ps, v_sb[:, c, :], e_cur[:, c, half],
                            start=(c == 0), stop=(c == CH - 1),
                        )
                        if emit_s:
                            s_exp(c, Gs + 1, e_tiles[Gs + 1])
                        if c < len(tail):
                            tail[c]()
                    if g % 2 == 1:
                        e_tiles.pop(Gs)
                    for j in qproj_sched.get(g, []):
                        q_proj_steady(j)
                    tail = make_tail(g, o_ps)
                for step in tail:
                    step()

                if taps_on:
                    def _tap(name, ap):
                        t32 = data.tile(list(ap.shape), f32, name="tap_" + name)
                        nc.vector.tensor_copy(t32, ap)
                        args = tuple(slice(None) for _ in ap.shape)
                        nc.gpsimd.dma_start(_DBG_PARAMS[name][args], t32)
                    _tap("dbg_qT", qT_sb)
                    _tap("dbg_kT", kT_sb)
                    _tap("dbg_vT", vT_sb)
                    _tap("dbg_v", v_sb)
                    _tap("dbg_kqb", kqb_sb)
                    _tap("dbg_ob2", ob2_sb)

    nc.finalize()
    return nc


def _get_nc():
    global _CACHED_NC
    if _CACHED_NC is None:
        _CACHED_NC = _build_nc()
    return _CACHED_NC


def make_in_maps(inputs):
    conv_feat = np.asarray(inputs["conv_feat"], dtype=np.float32)
    vit_feat = np.asarray(inputs["vit_feat"], dtype=np.float32)
    weights = {
        name: np.ascontiguousarray(np.asarray(inputs[name], dtype=np.float32))
        for name in ("q_w", "q_b", "k_w", "v_w", "v_b", "out_w", "out_b")
    }
    in_maps = []
    for b in range(B):
        m = dict(weights)
        m["conv_feat"] = np.ascontiguousarray(conv_feat[b].reshape(C, HW))
        m["vit_feat"] = np.ascontiguousarray(vit_feat[b])
        in_maps.append(m)
    return in_maps


def kernel(**inputs) -> np.ndarray:
    from concourse.bass_utils import run_bass_kernel_spmd

    nc = _get_nc()
    res = run_bass_kernel_spmd(nc, make_in_maps(inputs), list(range(B)))
    return np.stack(
        [res.results[b]["out"].reshape(C, H, W) for b in range(B)]
    ).astype(np.float32)
